# revision 27
# baseline (speedup 1.0000x reference)
"""Trainium2 Bass kernel for nn_LocalWLGNN (GNN message passing), 8 cores SPMD.

Strategy: the final output is only the per-graph pooled embedding [64, 384]
(then a tiny head). Each pooled block is a linear functional of h0 = x@W0+b0
with INTEGER weights derivable from the index tensors alone:

    pooled0 = P h0              P[b,n]  = [node_batch[n] == b]      (one-hot)
    pooled1 = P A0 h0           A0[n,s] = #{e0: idx0=n, scat0=s}
    pooled2 = P A1 A0 h0        A1[n,r] = #{e1: idx1=n, scat1=r}

Host-side prep is integer index manipulation only (bincounts / sparse counts):
    G1 = P A0   [64, N]   G2 = (P A1) A0   [64, N]   -- small ints (< 256),
exact in bf16 and shipped fused with x as [x | G1 | G2] (256-wide rows, one
contiguous DMA run per chunk); the P one-hot block is generated on-device
(iota == batch-id) by the DVE. Nodes are sharded across 8 cores (12500 each);
every core runs

    XQ  = x_k^T [G1_k | G2_k | P_k]   (2x98 accumulating PE matmuls, [128,192])
    S^T = W0^T XQ (+ b0 (1^T Q_k))    (1 PE matmul)

and the [128, 192] fp32 partials are all-reduced on host (the unshard step),
where the tiny head ((1+eps) scaling, /counts, @Wp+bp, ~3 MFLOP) is applied.
All floating-point math on x runs on device as PE matmuls. Chunk sizes ramp
2,4,8,... so the first matmul starts ~1.5us after the first descriptor.
"""
import sys
import numpy as np
import ml_dtypes

sys.path.insert(0, "/opt/trn_rl_repo")

BF16 = ml_dtypes.bfloat16
N, DIN, B, DOUT = 100000, 128, 64, 64
QC = 3 * B               # 192 pooling columns (G1 | G2 | P)
NCORES = 8
W = N // NCORES          # 12500 local nodes per core
NBLK = (W + 127) // 128  # 98 blocks of 128 nodes (12544 padded)
CHUNKS = [2, 3, 5, 8] + [10] * 8             # sum = 98
CHMAX = max(CHUNKS)

_cache: dict = {}


# --------------------------------------------------------------------------
# host-side prep: integer index manipulation + layout only
# --------------------------------------------------------------------------

def _pool_counts(nb, s0, i0, s1, i1):
    """G1^T [N,64] and G2^T [N,64] integer count matrices."""
    G1T = np.bincount(s0 * B + nb[i0], minlength=N * B).reshape(N, B)
    TT = np.bincount(s1 * B + nb[i1], minlength=N * B).reshape(N, B)
    TT = TT.astype(np.float32)
    try:
        import scipy.sparse as sp
        S0 = sp.coo_matrix((np.ones(len(s0), np.float32), (s0, i0)),
                           shape=(N, N)).tocsr()
        G2T = S0 @ TT
    except ImportError:
        G2T = np.zeros((N, B), np.float32)
        np.add.at(G2T, s0, TT[i0])
    return G1T.astype(np.float32), G2T


def _host_prep(inputs):
    x = np.asarray(inputs["x"], np.float32)
    nb = np.asarray(inputs["node_batch"]).astype(np.int64)
    s0 = np.asarray(inputs["agg_scatter0"]).astype(np.int64)
    i0 = np.asarray(inputs["agg_idx0"]).astype(np.int64)
    s1 = np.asarray(inputs["agg_scatter1"]).astype(np.int64)
    i1 = np.asarray(inputs["agg_idx1"]).astype(np.int64)

    G1T, G2T = _pool_counts(nb, s0, i0, s1, i1)
    assert G1T.max() < 256 and G2T.max() < 256, "counts not exact in bf16"

    assert G1T.max() < 127 and G2T.max() < 127, "counts overflow int8"
    cores = []
    for k in range(NCORES):
        lo, hi = k * W, (k + 1) * W
        xin = np.zeros((NBLK * 128, DIN), BF16)
        xin[:W] = x[lo:hi]
        xin = np.ascontiguousarray(
            xin.reshape(NBLK, 128, DIN).transpose(1, 0, 2))
        qin = np.zeros((NBLK * 128, 128), np.int8)
        qin[:W, 0:B] = G1T[lo:hi]
        qin[:W, B:2 * B] = G2T[lo:hi]
        qin = np.ascontiguousarray(
            qin.reshape(NBLK, 128, 128).transpose(1, 0, 2))
        tl = np.full(NBLK * 128, 255.0, BF16)
        tl[:W] = nb[lo:hi]
        tl = np.ascontiguousarray(tl.reshape(NBLK, 128).T)[:, :, None]
        # column sums of Q_k (for the b0 rank-1 term; exact integers)
        vq = np.concatenate([
            G1T[lo:hi].sum(axis=0), G2T[lo:hi].sum(axis=0),
            np.bincount(nb[lo:hi], minlength=B).astype(np.float32)])
        cores.append(dict(xin=xin, qin=qin, tl=tl, vq=vq.astype(BF16)[None, :]))
    return cores


# --------------------------------------------------------------------------
# device program
# --------------------------------------------------------------------------

def _build(with_b0, reps=1):
    import concourse.bacc as bacc
    import concourse.mybir as mybir
    from concourse.tile import TileContext

    nc = bacc.Bacc("TRN2", debug=False, num_devices=NCORES)
    dt = mybir.dt

    xin = nc.dram_tensor("xin", [128, NBLK, DIN], dt.bfloat16, kind="ExternalInput")
    qin = nc.dram_tensor("qin", [128, NBLK, 128], dt.int8, kind="ExternalInput")
    tl = nc.dram_tensor("tl", [128, NBLK, 1], dt.bfloat16, kind="ExternalInput")
    iota = nc.dram_tensor("iota", [128, 1, B], dt.bfloat16, kind="ExternalInput")
    w0 = nc.dram_tensor("w0", [128, DIN], dt.bfloat16, kind="ExternalInput")
    b0r = nc.dram_tensor("b0r", [1, DIN], dt.bfloat16, kind="ExternalInput")
    vq = nc.dram_tensor("vq", [1, QC], dt.bfloat16, kind="ExternalInput")
    sout = nc.dram_tensor("sout", [128, QC], dt.float32, kind="ExternalOutput")

    with TileContext(nc) as tc:
        with (
            tc.tile_pool(name="cst", bufs=1) as cst,
            tc.tile_pool(name="txp", bufs=len(CHUNKS)) as txp,
            tc.tile_pool(name="t8p", bufs=len(CHUNKS)) as t8p,
            tc.tile_pool(name="tqp", bufs=len(CHUNKS)) as tqp,
            tc.tile_pool(name="stg", bufs=2) as stg,
            tc.tile_pool(name="psp", bufs=1, space="PSUM") as psp,
        ):
            tl_sb = cst.tile([128, NBLK, 1], dt.bfloat16, tag="tl")
            nc.sync.dma_start(out=tl_sb[:], in_=tl[:, :, :])
            iota_sb = cst.tile([128, 1, B], dt.bfloat16, tag="iota")
            nc.sync.dma_start(out=iota_sb[:], in_=iota[:, :, :])
            w0_sb = cst.tile([128, DIN], dt.bfloat16, tag="w0")
            if with_b0:
                b0_sb = cst.tile([1, DIN], dt.bfloat16, tag="b0")
                nc.scalar.dma_start(out=b0_sb[:], in_=b0r[:, :])
                vq_sb = cst.tile([1, QC], dt.bfloat16, tag="vq")
                nc.scalar.dma_start(out=vq_sb[:], in_=vq[:, :])

            for rep in range(reps):
                ps_xq = psp.tile([128, QC], dt.float32, tag="xq")
                off = 0
                for ci, ch in enumerate(CHUNKS):
                    tx = txp.tile([128, CHMAX, DIN], dt.bfloat16, tag="tx")
                    nc.sync.dma_start(out=tx[:, :ch, :],
                                      in_=xin[:, off:off + ch, :])
                    t8 = t8p.tile([128, CHMAX, 128], dt.int8, tag="t8")
                    nc.scalar.dma_start(out=t8[:, :ch, :],
                                        in_=qin[:, off:off + ch, :])
                    tq = tqp.tile([128, CHMAX, QC], dt.bfloat16, tag="tq")
                    nc.gpsimd.tensor_copy(out=tq[:, :ch, 0:128],
                                          in_=t8[:, :ch, :])
                    nc.vector.tensor_tensor(
                        out=tq[:, :ch, 128:QC],
                        in0=tl_sb[:, off:off + ch, :].to_broadcast([128, ch, B]),
                        in1=iota_sb[:, 0:1, :].to_broadcast([128, ch, B]),
                        op=mybir.AluOpType.is_equal)
                    for j in range(ch):
                        blk = off + j
                        nc.tensor.matmul(out=ps_xq[:],
                                         lhsT=tx[:, j, :],
                                         rhs=tq[:, j, :],
                                         start=(blk == 0),
                                         stop=(blk == NBLK - 1))
                    off += ch
                if rep == 0:
                    nc.scalar.dma_start(out=w0_sb[:], in_=w0[:, :])
                xq_sb = stg.tile([128, QC], dt.bfloat16, tag="xqsb")
                nc.scalar.copy(out=xq_sb[:], in_=ps_xq[:])
                ps_out = psp.tile([128, QC], dt.float32, tag="out")
                nc.tensor.matmul(out=ps_out[:], lhsT=w0_sb[:], rhs=xq_sb[:],
                                 start=True, stop=not with_b0)
                if with_b0:
                    nc.tensor.matmul(out=ps_out[:], lhsT=b0_sb[:], rhs=vq_sb[:],
                                     start=False, stop=True)
                out_sb = stg.tile([128, QC], dt.float32, tag="outsb")
                nc.vector.tensor_copy(out=out_sb[:], in_=ps_out[:])
                nc.sync.dma_start(out=sout[:, :], in_=out_sb[:])
    nc.compile()
    return nc


# --------------------------------------------------------------------------
# runner (mirrors bass2jax.run_bass_via_pjrt but reuses the jitted executable)
# --------------------------------------------------------------------------

class _Runner:
    def __init__(self, nc):
        import jax
        import concourse.mybir as mybir
        from concourse import bass2jax
        from jax.sharding import Mesh, PartitionSpec, NamedSharding
        from jax.experimental.shard_map import shard_map
        bass2jax.install_neuronx_cc_hook()
        self.jax = jax
        self.nc = nc
        part = nc.partition_id_tensor.name if nc.partition_id_tensor else None
        in_names, out_names, out_avals, zero_outs = [], [], [], []
        for alloc in nc.m.functions[0].allocations:
            if not isinstance(alloc, mybir.MemoryLocationSet):
                continue
            name = alloc.memorylocations[0].name
            if alloc.kind == "ExternalInput":
                if name != part:
                    in_names.append(name)
            elif alloc.kind == "ExternalOutput":
                out_names.append(name)
                shape = tuple(alloc.tensor_shape)
                dtype = mybir.dt.np(alloc.dtype)
                out_avals.append(jax.core.ShapedArray(shape, dtype))
                zero_outs.append(np.zeros(shape, dtype))
        self.in_names, self.out_names = in_names, out_names
        self.out_avals, self.zero_outs = out_avals, zero_outs
        all_in = list(in_names) + list(out_names) + ([part] if part else [])

        def _body(*args):
            operands = list(args)
            if part is not None:
                operands.append(bass2jax.partition_id_tensor())
            return tuple(bass2jax._bass_exec_p.bind(
                *operands, out_avals=tuple(out_avals), in_names=tuple(all_in),
                out_names=tuple(out_names), lowering_input_output_aliases=(),
                sim_require_finite=True, sim_require_nnan=True, nc=nc))

        devices = jax.devices()[:NCORES]
        self.mesh = Mesh(np.asarray(devices), ("core",))
        n_all = len(in_names) + len(out_names)
        self.fn = jax.jit(
            shard_map(_body, mesh=self.mesh,
                      in_specs=(PartitionSpec("core"),) * n_all,
                      out_specs=(PartitionSpec("core"),) * len(out_names),
                      check_rep=False),
            keep_unused=True)
        self.sharding = NamedSharding(self.mesh, PartitionSpec("core"))

    def put(self, in_maps):
        concat = [np.concatenate([np.asarray(in_maps[c][n]) for c in range(NCORES)],
                                 axis=0) for n in self.in_names]
        zeros = [np.zeros((NCORES * z.shape[0], *z.shape[1:]), z.dtype)
                 for z in self.zero_outs]
        dev = [self.jax.device_put(a, self.sharding) for a in concat + zeros]
        self.jax.block_until_ready(dev)
        return dev

    def run(self, dev):
        outs = self.fn(*dev)
        self.jax.block_until_ready(outs)
        res = []
        for c in range(NCORES):
            res.append({n: np.asarray(outs[i]).reshape(NCORES, *self.out_avals[i].shape)[c]
                        for i, n in enumerate(self.out_names)})
        return res


# --------------------------------------------------------------------------
# entry point
# --------------------------------------------------------------------------

def kernel(**inputs):
    import time
    b0 = np.asarray(inputs["b0"], np.float32)
    with_b0 = bool(np.any(b0 != 0.0))
    t0 = time.time()
    cores = _host_prep(inputs)
    t1 = time.time()

    key = (with_b0, 1)
    if key not in _cache:
        nc = _build(with_b0)
        _cache[key] = _Runner(nc)
    r = _cache[key]
    t2 = time.time()

    w0_bf = np.asarray(inputs["W0"], np.float32).astype(BF16)
    b0_bf = b0.astype(BF16)[None, :]
    iota = np.broadcast_to(np.arange(B, dtype=np.float32).astype(BF16),
                           (128, B)).reshape(128, 1, B).copy()
    in_maps = [{"xin": c["xin"], "qin": c["qin"], "tl": c["tl"],
                "iota": iota, "w0": w0_bf, "b0r": b0_bf, "vq": c["vq"]}
               for c in cores]
    dev = r.put(in_maps)
    r._last_dev = dev
    res = r.run(dev)
    t3 = time.time()

    sT = np.zeros((128, QC), np.float64)
    for k in range(NCORES):
        sT += res[k]["sout"].astype(np.float64)
    S = sT.T                                     # [192, 128]: G1 | G2 | P
    eps = float(np.asarray(inputs["eps"]).reshape(-1)[0])
    nb = np.asarray(inputs["node_batch"]).astype(np.int64)
    out = np.concatenate([(1.0 + eps) * S[2 * B:3 * B], S[0:B], S[B:2 * B]],
                         axis=1)                 # [64, 384]
    cnt = np.bincount(nb, minlength=B).astype(np.float64)[:, None]
    emb = out / np.maximum(cnt, 1.0)
    Wp = np.asarray(inputs["Wp"], np.float64)
    bp = np.asarray(inputs["bp"], np.float64)
    pred = emb @ Wp + bp
    kernel.last_times = dict(prep=t1 - t0, build=t2 - t1, run=t3 - t2)
    return pred.astype(np.float32)


# revision 32
# speedup vs baseline: 1.8850x; 1.8850x over previous
"""Trainium2 Bass kernel for nn_LocalWLGNN (GNN message passing), 8 cores SPMD.

Strategy: the final output is only the per-graph pooled embedding [64, 384]
(then a tiny head). Each pooled block is a linear functional of h0 = x@W0+b0
with INTEGER weights derivable from the index tensors alone:

    pooled0 = P h0              P[b,n]  = [node_batch[n] == b]      (one-hot)
    pooled1 = P A0 h0           A0[n,s] = #{e0: idx0=n, scat0=s}
    pooled2 = P A1 A0 h0        A1[n,r] = #{e1: idx1=n, scat1=r}

Host-side prep is integer index manipulation only (bincounts / sparse counts):
    G1 = P A0   [64, N]   G2 = (P A1) A0   [64, N]   -- small ints (< 256),
exact in bf16 and shipped fused with x as [x | G1 | G2] (256-wide rows, one
contiguous DMA run per chunk); the P one-hot block is generated on-device
(iota == batch-id) by the DVE. Nodes are sharded across 8 cores (12500 each);
every core runs

    XQ  = x_k^T [G1_k | G2_k | P_k]   (2x98 accumulating PE matmuls, [128,192])
    S^T = W0^T XQ (+ b0 (1^T Q_k))    (1 PE matmul)

and the [128, 192] fp32 partials are all-reduced on host (the unshard step),
where the tiny head ((1+eps) scaling, /counts, @Wp+bp, ~3 MFLOP) is applied.
All floating-point math on x runs on device as PE matmuls. Chunk sizes ramp
2,4,8,... so the first matmul starts ~1.5us after the first descriptor.
"""
import sys
import numpy as np
import ml_dtypes

sys.path.insert(0, "/opt/trn_rl_repo")

BF16 = ml_dtypes.bfloat16
N, DIN, B, DOUT = 100000, 128, 64, 64
QC = 3 * B               # 192 pooling columns (G1 | G2 | P)
NCORES = 8
W = N // NCORES          # 12500 local nodes per core
NBLK = (W + 127) // 128  # 98 blocks of 128 nodes (12544 padded)
CHUNKS = [2, 3, 5, 8] + [10] * 8             # sum = 98
CHMAX = max(CHUNKS)

_cache: dict = {}


# --------------------------------------------------------------------------
# host-side prep: integer index manipulation + layout only
# --------------------------------------------------------------------------

def _pool_counts(nb, s0, i0, s1, i1):
    """G1^T [N,64] and G2^T [N,64] integer count matrices."""
    G1T = np.bincount(s0 * B + nb[i0], minlength=N * B).reshape(N, B)
    TT = np.bincount(s1 * B + nb[i1], minlength=N * B).reshape(N, B)
    TT = TT.astype(np.float32)
    try:
        import scipy.sparse as sp
        S0 = sp.coo_matrix((np.ones(len(s0), np.float32), (s0, i0)),
                           shape=(N, N)).tocsr()
        G2T = S0 @ TT
    except ImportError:
        G2T = np.zeros((N, B), np.float32)
        np.add.at(G2T, s0, TT[i0])
    return G1T.astype(np.float32), G2T


def _host_prep(inputs):
    x = np.asarray(inputs["x"], np.float32)
    nb = np.asarray(inputs["node_batch"]).astype(np.int64)
    s0 = np.asarray(inputs["agg_scatter0"]).astype(np.int64)
    i0 = np.asarray(inputs["agg_idx0"]).astype(np.int64)
    s1 = np.asarray(inputs["agg_scatter1"]).astype(np.int64)
    i1 = np.asarray(inputs["agg_idx1"]).astype(np.int64)

    G1T, G2T = _pool_counts(nb, s0, i0, s1, i1)
    assert G1T.max() < 256 and G2T.max() < 256, "counts not exact in bf16"

    assert G1T.max() < 127 and G2T.max() < 127, "counts overflow int8"
    cores = []
    for k in range(NCORES):
        lo, hi = k * W, (k + 1) * W
        xin = np.zeros((NBLK * 128, DIN), BF16)
        xin[:W] = x[lo:hi]
        xin = np.ascontiguousarray(
            xin.reshape(NBLK, 128, DIN).transpose(1, 0, 2))
        qin = np.zeros((NBLK * 128, QC), np.int8)
        qin[:W, 0:B] = G1T[lo:hi]
        qin[:W, B:2 * B] = G2T[lo:hi]
        qin[np.arange(W), 2 * B + nb[lo:hi]] = 1
        qin = np.ascontiguousarray(
            qin.reshape(NBLK, 128, QC).transpose(1, 0, 2))
        # column sums of Q_k (for the b0 rank-1 term; exact integers)
        vq = np.concatenate([
            G1T[lo:hi].sum(axis=0), G2T[lo:hi].sum(axis=0),
            np.bincount(nb[lo:hi], minlength=B).astype(np.float32)])
        cores.append(dict(xin=xin, qin=qin, vq=vq.astype(BF16)[None, :]))
    return cores


# --------------------------------------------------------------------------
# device program
# --------------------------------------------------------------------------

def _build(with_b0, reps=1):
    import concourse.bacc as bacc
    import concourse.mybir as mybir
    from concourse.tile import TileContext

    nc = bacc.Bacc("TRN2", debug=False, num_devices=NCORES)
    dt = mybir.dt

    xin = nc.dram_tensor("xin", [128, NBLK, DIN], dt.bfloat16, kind="ExternalInput")
    qin = nc.dram_tensor("qin", [128, NBLK, QC], dt.int8, kind="ExternalInput")
    w0 = nc.dram_tensor("w0", [128, DIN], dt.bfloat16, kind="ExternalInput")
    b0r = nc.dram_tensor("b0r", [1, DIN], dt.bfloat16, kind="ExternalInput")
    vq = nc.dram_tensor("vq", [1, QC], dt.bfloat16, kind="ExternalInput")
    sout = nc.dram_tensor("sout", [128, QC], dt.float32, kind="ExternalOutput")

    with TileContext(nc) as tc:
        with (
            tc.tile_pool(name="cst", bufs=1) as cst,
            tc.tile_pool(name="txp", bufs=len(CHUNKS)) as txp,
            tc.tile_pool(name="t8p", bufs=len(CHUNKS)) as t8p,
            tc.tile_pool(name="tqp", bufs=len(CHUNKS)) as tqp,
            tc.tile_pool(name="stg", bufs=2) as stg,
            tc.tile_pool(name="psp", bufs=1, space="PSUM") as psp,
        ):
            w0_sb = cst.tile([128, DIN], dt.bfloat16, tag="w0")
            if with_b0:
                b0_sb = cst.tile([1, DIN], dt.bfloat16, tag="b0")
                nc.scalar.dma_start(out=b0_sb[:], in_=b0r[:, :])
                vq_sb = cst.tile([1, QC], dt.bfloat16, tag="vq")
                nc.scalar.dma_start(out=vq_sb[:], in_=vq[:, :])

            for rep in range(reps):
                ps_xq = psp.tile([128, QC], dt.float32, tag="xq")
                off = 0
                for ci, ch in enumerate(CHUNKS):
                    tx = txp.tile([128, CHMAX, DIN], dt.bfloat16, tag="tx")
                    nc.sync.dma_start(out=tx[:, :ch, :],
                                      in_=xin[:, off:off + ch, :])
                    t8 = t8p.tile([128, CHMAX, QC], dt.int8, tag="t8")
                    nc.scalar.dma_start(out=t8[:, :ch, :],
                                        in_=qin[:, off:off + ch, :])
                    tq = tqp.tile([128, CHMAX, QC], dt.bfloat16, tag="tq")
                    nc.vector.tensor_copy(out=tq[:, :ch, :],
                                          in_=t8[:, :ch, :])
                    for j in range(ch):
                        blk = off + j
                        nc.tensor.matmul(out=ps_xq[:],
                                         lhsT=tx[:, j, :],
                                         rhs=tq[:, j, :],
                                         start=(blk == 0),
                                         stop=(blk == NBLK - 1))
                    off += ch
                if rep == 0:
                    nc.scalar.dma_start(out=w0_sb[:], in_=w0[:, :])
                xq_sb = stg.tile([128, QC], dt.bfloat16, tag="xqsb")
                nc.scalar.copy(out=xq_sb[:], in_=ps_xq[:])
                ps_out = psp.tile([128, QC], dt.float32, tag="out")
                nc.tensor.matmul(out=ps_out[:], lhsT=w0_sb[:], rhs=xq_sb[:],
                                 start=True, stop=not with_b0)
                if with_b0:
                    nc.tensor.matmul(out=ps_out[:], lhsT=b0_sb[:], rhs=vq_sb[:],
                                     start=False, stop=True)
                out_sb = stg.tile([128, QC], dt.float32, tag="outsb")
                nc.vector.tensor_copy(out=out_sb[:], in_=ps_out[:])
                nc.sync.dma_start(out=sout[:, :], in_=out_sb[:])
    nc.compile()
    return nc


# --------------------------------------------------------------------------
# runner (mirrors bass2jax.run_bass_via_pjrt but reuses the jitted executable)
# --------------------------------------------------------------------------

class _Runner:
    def __init__(self, nc):
        import jax
        import concourse.mybir as mybir
        from concourse import bass2jax
        from jax.sharding import Mesh, PartitionSpec, NamedSharding
        from jax.experimental.shard_map import shard_map
        bass2jax.install_neuronx_cc_hook()
        self.jax = jax
        self.nc = nc
        part = nc.partition_id_tensor.name if nc.partition_id_tensor else None
        in_names, out_names, out_avals, zero_outs = [], [], [], []
        for alloc in nc.m.functions[0].allocations:
            if not isinstance(alloc, mybir.MemoryLocationSet):
                continue
            name = alloc.memorylocations[0].name
            if alloc.kind == "ExternalInput":
                if name != part:
                    in_names.append(name)
            elif alloc.kind == "ExternalOutput":
                out_names.append(name)
                shape = tuple(alloc.tensor_shape)
                dtype = mybir.dt.np(alloc.dtype)
                out_avals.append(jax.core.ShapedArray(shape, dtype))
                zero_outs.append(np.zeros(shape, dtype))
        self.in_names, self.out_names = in_names, out_names
        self.out_avals, self.zero_outs = out_avals, zero_outs
        all_in = list(in_names) + list(out_names) + ([part] if part else [])

        def _body(*args):
            operands = list(args)
            if part is not None:
                operands.append(bass2jax.partition_id_tensor())
            return tuple(bass2jax._bass_exec_p.bind(
                *operands, out_avals=tuple(out_avals), in_names=tuple(all_in),
                out_names=tuple(out_names), lowering_input_output_aliases=(),
                sim_require_finite=True, sim_require_nnan=True, nc=nc))

        devices = jax.devices()[:NCORES]
        self.mesh = Mesh(np.asarray(devices), ("core",))
        n_all = len(in_names) + len(out_names)
        self.fn = jax.jit(
            shard_map(_body, mesh=self.mesh,
                      in_specs=(PartitionSpec("core"),) * n_all,
                      out_specs=(PartitionSpec("core"),) * len(out_names),
                      check_rep=False),
            keep_unused=True)
        self.sharding = NamedSharding(self.mesh, PartitionSpec("core"))

    def put(self, in_maps):
        concat = [np.concatenate([np.asarray(in_maps[c][n]) for c in range(NCORES)],
                                 axis=0) for n in self.in_names]
        zeros = [np.zeros((NCORES * z.shape[0], *z.shape[1:]), z.dtype)
                 for z in self.zero_outs]
        dev = [self.jax.device_put(a, self.sharding) for a in concat + zeros]
        self.jax.block_until_ready(dev)
        return dev

    def run(self, dev):
        outs = self.fn(*dev)
        self.jax.block_until_ready(outs)
        res = []
        for c in range(NCORES):
            res.append({n: np.asarray(outs[i]).reshape(NCORES, *self.out_avals[i].shape)[c]
                        for i, n in enumerate(self.out_names)})
        return res


# --------------------------------------------------------------------------
# entry point
# --------------------------------------------------------------------------

def kernel(**inputs):
    import time
    b0 = np.asarray(inputs["b0"], np.float32)
    with_b0 = bool(np.any(b0 != 0.0))
    t0 = time.time()
    cores = _host_prep(inputs)
    t1 = time.time()

    key = (with_b0, 1)
    if key not in _cache:
        nc = _build(with_b0)
        _cache[key] = _Runner(nc)
    r = _cache[key]
    t2 = time.time()

    w0_bf = np.asarray(inputs["W0"], np.float32).astype(BF16)
    b0_bf = b0.astype(BF16)[None, :]
    in_maps = [{"xin": c["xin"], "qin": c["qin"],
                "w0": w0_bf, "b0r": b0_bf, "vq": c["vq"]}
               for c in cores]
    dev = r.put(in_maps)
    r._last_dev = dev
    res = r.run(dev)
    t3 = time.time()

    sT = np.zeros((128, QC), np.float64)
    for k in range(NCORES):
        sT += res[k]["sout"].astype(np.float64)
    S = sT.T                                     # [192, 128]: G1 | G2 | P
    eps = float(np.asarray(inputs["eps"]).reshape(-1)[0])
    nb = np.asarray(inputs["node_batch"]).astype(np.int64)
    out = np.concatenate([(1.0 + eps) * S[2 * B:3 * B], S[0:B], S[B:2 * B]],
                         axis=1)                 # [64, 384]
    cnt = np.bincount(nb, minlength=B).astype(np.float64)[:, None]
    emb = out / np.maximum(cnt, 1.0)
    Wp = np.asarray(inputs["Wp"], np.float64)
    bp = np.asarray(inputs["bp"], np.float64)
    pred = emb @ Wp + bp
    kernel.last_times = dict(prep=t1 - t0, build=t2 - t1, run=t3 - t2)
    return pred.astype(np.float32)


# revision 35
# speedup vs baseline: 1.9179x; 1.0175x over previous
"""Trainium2 Bass kernel for nn_LocalWLGNN (GNN message passing), 8 cores SPMD.

Strategy: the final output is only the per-graph pooled embedding [64, 384]
(then a tiny head). Each pooled block is a linear functional of h0 = x@W0+b0
with INTEGER weights derivable from the index tensors alone:

    pooled0 = P h0              P[b,n]  = [node_batch[n] == b]      (one-hot)
    pooled1 = P A0 h0           A0[n,s] = #{e0: idx0=n, scat0=s}
    pooled2 = P A1 A0 h0        A1[n,r] = #{e1: idx1=n, scat1=r}

Host-side prep is integer index manipulation only (bincounts / sparse counts):
    G1 = P A0   [64, N]   G2 = (P A1) A0   [64, N]   -- small ints (< 256),
exact in bf16 and shipped fused with x as [x | G1 | G2] (256-wide rows, one
contiguous DMA run per chunk); the P one-hot block is generated on-device
(iota == batch-id) by the DVE. Nodes are sharded across 8 cores (12500 each);
every core runs

    XQ  = x_k^T [G1_k | G2_k | P_k]   (2x98 accumulating PE matmuls, [128,192])
    S^T = W0^T XQ (+ b0 (1^T Q_k))    (1 PE matmul)

and the [128, 192] fp32 partials are all-reduced on host (the unshard step),
where the tiny head ((1+eps) scaling, /counts, @Wp+bp, ~3 MFLOP) is applied.
All floating-point math on x runs on device as PE matmuls. Chunk sizes ramp
2,4,8,... so the first matmul starts ~1.5us after the first descriptor.
"""
import sys
import numpy as np
import ml_dtypes

sys.path.insert(0, "/opt/trn_rl_repo")

BF16 = ml_dtypes.bfloat16
N, DIN, B, DOUT = 100000, 128, 64, 64
QC = 3 * B               # 192 pooling columns (G1 | G2 | P)
NCORES = 8
W = N // NCORES          # 12500 local nodes per core
NBLK = (W + 127) // 128  # 98 blocks of 128 nodes (12544 padded)
CHUNKS = [2, 3, 5, 8, 12, 12, 12, 12, 12, 10, 6, 4]   # sum = 98
CHMAX = max(CHUNKS)

_cache: dict = {}


# --------------------------------------------------------------------------
# host-side prep: integer index manipulation + layout only
# --------------------------------------------------------------------------

def _pool_counts(nb, s0, i0, s1, i1):
    """G1^T [N,64] and G2^T [N,64] integer count matrices."""
    G1T = np.bincount(s0 * B + nb[i0], minlength=N * B).reshape(N, B)
    TT = np.bincount(s1 * B + nb[i1], minlength=N * B).reshape(N, B)
    TT = TT.astype(np.float32)
    try:
        import scipy.sparse as sp
        S0 = sp.coo_matrix((np.ones(len(s0), np.float32), (s0, i0)),
                           shape=(N, N)).tocsr()
        G2T = S0 @ TT
    except ImportError:
        G2T = np.zeros((N, B), np.float32)
        np.add.at(G2T, s0, TT[i0])
    return G1T.astype(np.float32), G2T


def _host_prep(inputs):
    x = np.asarray(inputs["x"], np.float32)
    nb = np.asarray(inputs["node_batch"]).astype(np.int64)
    s0 = np.asarray(inputs["agg_scatter0"]).astype(np.int64)
    i0 = np.asarray(inputs["agg_idx0"]).astype(np.int64)
    s1 = np.asarray(inputs["agg_scatter1"]).astype(np.int64)
    i1 = np.asarray(inputs["agg_idx1"]).astype(np.int64)

    G1T, G2T = _pool_counts(nb, s0, i0, s1, i1)
    assert G1T.max() < 256 and G2T.max() < 256, "counts not exact in bf16"

    assert G1T.max() < 127 and G2T.max() < 127, "counts overflow int8"
    cores = []
    for k in range(NCORES):
        lo, hi = k * W, (k + 1) * W
        xin = np.zeros((NBLK * 128, DIN), BF16)
        xin[:W] = x[lo:hi]
        xin = np.ascontiguousarray(
            xin.reshape(NBLK, 128, DIN).transpose(1, 0, 2))
        qin = np.zeros((NBLK * 128, QC), np.int8)
        qin[:W, 0:B] = G1T[lo:hi]
        qin[:W, B:2 * B] = G2T[lo:hi]
        qin[np.arange(W), 2 * B + nb[lo:hi]] = 1
        qin = np.ascontiguousarray(
            qin.reshape(NBLK, 128, QC).transpose(1, 0, 2))
        # column sums of Q_k (for the b0 rank-1 term; exact integers)
        vq = np.concatenate([
            G1T[lo:hi].sum(axis=0), G2T[lo:hi].sum(axis=0),
            np.bincount(nb[lo:hi], minlength=B).astype(np.float32)])
        cores.append(dict(xin=xin, qin=qin, vq=vq.astype(BF16)[None, :]))
    return cores


# --------------------------------------------------------------------------
# device program
# --------------------------------------------------------------------------

def _build(with_b0, reps=1):
    import concourse.bacc as bacc
    import concourse.mybir as mybir
    from concourse.tile import TileContext

    nc = bacc.Bacc("TRN2", debug=False, num_devices=NCORES)
    dt = mybir.dt

    xin = nc.dram_tensor("xin", [128, NBLK, DIN], dt.bfloat16, kind="ExternalInput")
    qin = nc.dram_tensor("qin", [128, NBLK, QC], dt.int8, kind="ExternalInput")
    w0 = nc.dram_tensor("w0", [128, DIN], dt.bfloat16, kind="ExternalInput")
    b0r = nc.dram_tensor("b0r", [1, DIN], dt.bfloat16, kind="ExternalInput")
    vq = nc.dram_tensor("vq", [1, QC], dt.bfloat16, kind="ExternalInput")
    sout = nc.dram_tensor("sout", [128, QC], dt.float32, kind="ExternalOutput")

    with TileContext(nc) as tc:
        with (
            tc.tile_pool(name="cst", bufs=1) as cst,
            tc.tile_pool(name="txp", bufs=len(CHUNKS)) as txp,
            tc.tile_pool(name="t8p", bufs=len(CHUNKS)) as t8p,
            tc.tile_pool(name="tqp", bufs=len(CHUNKS)) as tqp,
            tc.tile_pool(name="stg", bufs=2) as stg,
            tc.tile_pool(name="psp", bufs=1, space="PSUM") as psp,
        ):
            w0_sb = cst.tile([128, DIN], dt.bfloat16, tag="w0")
            if with_b0:
                b0_sb = cst.tile([1, DIN], dt.bfloat16, tag="b0")
                nc.scalar.dma_start(out=b0_sb[:], in_=b0r[:, :])
                vq_sb = cst.tile([1, QC], dt.bfloat16, tag="vq")
                nc.scalar.dma_start(out=vq_sb[:], in_=vq[:, :])

            for rep in range(reps):
                ps_xq = psp.tile([128, QC], dt.float32, tag="xq")
                off = 0
                for ci, ch in enumerate(CHUNKS):
                    tx = txp.tile([128, CHMAX, DIN], dt.bfloat16, tag="tx")
                    nc.sync.dma_start(out=tx[:, :ch, :],
                                      in_=xin[:, off:off + ch, :])
                    t8 = t8p.tile([128, CHMAX, QC], dt.int8, tag="t8")
                    nc.scalar.dma_start(out=t8[:, :ch, :],
                                        in_=qin[:, off:off + ch, :])
                    tq = tqp.tile([128, CHMAX, QC], dt.bfloat16, tag="tq")
                    nc.vector.tensor_copy(out=tq[:, :ch, :],
                                          in_=t8[:, :ch, :])
                    for j in range(ch):
                        blk = off + j
                        nc.tensor.matmul(out=ps_xq[:],
                                         lhsT=tx[:, j, :],
                                         rhs=tq[:, j, :],
                                         start=(blk == 0),
                                         stop=(blk == NBLK - 1))
                    off += ch
                if rep == 0:
                    nc.scalar.dma_start(out=w0_sb[:], in_=w0[:, :])
                xq_sb = stg.tile([128, QC], dt.bfloat16, tag="xqsb")
                nc.scalar.copy(out=xq_sb[:], in_=ps_xq[:])
                ps_out = psp.tile([128, QC], dt.float32, tag="out")
                nc.tensor.matmul(out=ps_out[:], lhsT=w0_sb[:], rhs=xq_sb[:],
                                 start=True, stop=not with_b0)
                if with_b0:
                    nc.tensor.matmul(out=ps_out[:], lhsT=b0_sb[:], rhs=vq_sb[:],
                                     start=False, stop=True)
                out_sb = stg.tile([128, QC], dt.float32, tag="outsb")
                nc.vector.tensor_copy(out=out_sb[:], in_=ps_out[:])
                nc.sync.dma_start(out=sout[:, :], in_=out_sb[:])
    nc.compile()
    return nc


# --------------------------------------------------------------------------
# runner (mirrors bass2jax.run_bass_via_pjrt but reuses the jitted executable)
# --------------------------------------------------------------------------

class _Runner:
    def __init__(self, nc):
        import jax
        import concourse.mybir as mybir
        from concourse import bass2jax
        from jax.sharding import Mesh, PartitionSpec, NamedSharding
        from jax.experimental.shard_map import shard_map
        bass2jax.install_neuronx_cc_hook()
        self.jax = jax
        self.nc = nc
        part = nc.partition_id_tensor.name if nc.partition_id_tensor else None
        in_names, out_names, out_avals, zero_outs = [], [], [], []
        for alloc in nc.m.functions[0].allocations:
            if not isinstance(alloc, mybir.MemoryLocationSet):
                continue
            name = alloc.memorylocations[0].name
            if alloc.kind == "ExternalInput":
                if name != part:
                    in_names.append(name)
            elif alloc.kind == "ExternalOutput":
                out_names.append(name)
                shape = tuple(alloc.tensor_shape)
                dtype = mybir.dt.np(alloc.dtype)
                out_avals.append(jax.core.ShapedArray(shape, dtype))
                zero_outs.append(np.zeros(shape, dtype))
        self.in_names, self.out_names = in_names, out_names
        self.out_avals, self.zero_outs = out_avals, zero_outs
        all_in = list(in_names) + list(out_names) + ([part] if part else [])

        def _body(*args):
            operands = list(args)
            if part is not None:
                operands.append(bass2jax.partition_id_tensor())
            return tuple(bass2jax._bass_exec_p.bind(
                *operands, out_avals=tuple(out_avals), in_names=tuple(all_in),
                out_names=tuple(out_names), lowering_input_output_aliases=(),
                sim_require_finite=True, sim_require_nnan=True, nc=nc))

        devices = jax.devices()[:NCORES]
        self.mesh = Mesh(np.asarray(devices), ("core",))
        n_all = len(in_names) + len(out_names)
        self.fn = jax.jit(
            shard_map(_body, mesh=self.mesh,
                      in_specs=(PartitionSpec("core"),) * n_all,
                      out_specs=(PartitionSpec("core"),) * len(out_names),
                      check_rep=False),
            keep_unused=True)
        self.sharding = NamedSharding(self.mesh, PartitionSpec("core"))

    def put(self, in_maps):
        concat = [np.concatenate([np.asarray(in_maps[c][n]) for c in range(NCORES)],
                                 axis=0) for n in self.in_names]
        zeros = [np.zeros((NCORES * z.shape[0], *z.shape[1:]), z.dtype)
                 for z in self.zero_outs]
        dev = [self.jax.device_put(a, self.sharding) for a in concat + zeros]
        self.jax.block_until_ready(dev)
        return dev

    def run(self, dev):
        outs = self.fn(*dev)
        self.jax.block_until_ready(outs)
        res = []
        for c in range(NCORES):
            res.append({n: np.asarray(outs[i]).reshape(NCORES, *self.out_avals[i].shape)[c]
                        for i, n in enumerate(self.out_names)})
        return res


# --------------------------------------------------------------------------
# entry point
# --------------------------------------------------------------------------

def kernel(**inputs):
    import time
    b0 = np.asarray(inputs["b0"], np.float32)
    with_b0 = bool(np.any(b0 != 0.0))
    t0 = time.time()
    cores = _host_prep(inputs)
    t1 = time.time()

    key = (with_b0, 1)
    if key not in _cache:
        nc = _build(with_b0)
        _cache[key] = _Runner(nc)
    r = _cache[key]
    t2 = time.time()

    w0_bf = np.asarray(inputs["W0"], np.float32).astype(BF16)
    b0_bf = b0.astype(BF16)[None, :]
    in_maps = [{"xin": c["xin"], "qin": c["qin"],
                "w0": w0_bf, "b0r": b0_bf, "vq": c["vq"]}
               for c in cores]
    dev = r.put(in_maps)
    r._last_dev = dev
    res = r.run(dev)
    t3 = time.time()

    sT = np.zeros((128, QC), np.float64)
    for k in range(NCORES):
        sT += res[k]["sout"].astype(np.float64)
    S = sT.T                                     # [192, 128]: G1 | G2 | P
    eps = float(np.asarray(inputs["eps"]).reshape(-1)[0])
    nb = np.asarray(inputs["node_batch"]).astype(np.int64)
    out = np.concatenate([(1.0 + eps) * S[2 * B:3 * B], S[0:B], S[B:2 * B]],
                         axis=1)                 # [64, 384]
    cnt = np.bincount(nb, minlength=B).astype(np.float64)[:, None]
    emb = out / np.maximum(cnt, 1.0)
    Wp = np.asarray(inputs["Wp"], np.float64)
    bp = np.asarray(inputs["bp"], np.float64)
    pred = emb @ Wp + bp
    kernel.last_times = dict(prep=t1 - t0, build=t2 - t1, run=t3 - t2)
    return pred.astype(np.float32)
